# revision 27
# baseline (speedup 1.0000x reference)
"""CostVolume2D Trainium2 kernel (v3, ~114us HW vs 199us baseline).

out[b, d, h, w] = mean_c l[b,c,h,w] * r[b,c,h, w - (d - maxd)]   (zero padded)

Strategy (8 NeuronCores, shard H — no halo since shifts only touch W).
The kernel is HBM/DMA-bound: ~16 DMA engines x ~22.5 B/ns; every design
choice below minimizes bytes moved and keeps descriptors >= 512B.

  * Per (b, h): the 97 disparity planes are the diagonals of banded gram
    blocks G_q[i, n] = sum_c l[c, 128q+i] r[c, 128q-48+n], n in [0,224).
  * Tensor engine: 4 matmuls per h (K=64 channels, M=128 w's, N<=224).
    Each q-PAIR's matmuls write ONE single-bank PSUM tile [128, 448 f32]
    INTERLEAVED (psum col = 2n + qq, out-AP free stride 2). Interleaved
    matmul PSUM writes are correct as long as no write crosses a 2KB
    PSUM bank boundary (2-bank-spanning interleave-4 writes corrupt).
  * Eviction (PSUM -> SBUF f32->f16) is then a plain CONTIGUOUS copy per
    q-pair into g block tb = 2t+qp (448 cols per block, 8 blocks per
    4-h group tile). Split DVE (qp=0) / Activation (qp=1), ~615ns each.
    (Strided DVE/Act WRITES cost 2x — that's why the matmul, not the
    eviction, does the interleaving.)
  * Store: in an interleave-2 block, row i's 194 valid values (97-band
    of a q-pair) sit at cols [2i, 2i+194). For a 32-row sub-block m they
    all fall inside cols [64m, 64m+256) — a plain rectangular slice.
    One 3-dim DMA per (group, m) writes [32 rows x 8 blocks x 256] with
    512B descriptors: 16.8 MB/core vs 29.4 MB for the full-gram store
    (valid data is 12.7 MB). R=32/512B is the optimum under the
    "descriptors < 512B pay 2x" rule + ~10ns fixed cost per descriptor.
  * Input: rows packed [l 512 | r 512] f16, NO zero pad: the edge
    matmuls (q=0/3) use shortened rhs windows (N=176) and the host
    zeroes the out-of-image disparity outputs (stale PSUM) afterwards.
    Each partition (hh, c) reads its 16 h-rows contiguously; loads are
    issued as EIGHTHS on gpsimd (SWDGE, ~25ns dispatch vs ~600ns on
    SP/Act sequencers), one per h-group, prefetching b+1 during b —
    emission order IS issue order per engine, so loads must be emitted
    ahead and spread to avoid saturating DMA in bursts.
  * Store issue is split SP (3/8) / gpsimd (5/8) for the same
    sequencer-cost reason.
  * Host pre-divides l by C (exact, power of two); host unshard is a
    strided view + transpose + edge zeroing (pure layout glue).
"""

import sys

try:
    import concourse  # noqa: F401
except ImportError:
    sys.path.insert(0, "/opt/trn_rl_repo")

import numpy as np

from concourse import bass, mybir
from concourse import tile
from concourse.ap import AP
from concourse.bass_utils import run_bass_kernel_spmd

F32 = mybir.dt.float32
F16 = mybir.dt.float16

# Problem dims (hardcoded per spec)
B, C, H, W = 4, 64, 256, 512
MAXD = 48
D = 2 * MAXD + 1          # 97 disparity planes
NCORES = 8
HS = H // NCORES          # 32 h-rows per core

# Tiling
NH4 = HS // 2             # 16 h-pairs per core (partition dim packs hh in 2)
WROW = 2 * W              # 1024: [l 512 | r 512] per (c, h) row (no zero pad:
                          # edge matmuls are shortened and the host zeroes
                          # the out-of-image disparities afterwards)
NHG = HS // 4             # 8 groups of 4 h per g-tile
GPW = 4 * 224             # 896 interleaved gram columns per h
NM = 4                    # 32-row store sub-blocks
SBW = 512                 # stored row width per sub-block (388 valid + skew)

LAST_RESULTS = None
_NC_CACHE = {}


def _build_nc():
    nc = bass.Bass()
    lr_in = nc.dram_tensor("lr", [B, 2, C, NH4, WROW], F16, kind="ExternalInput")
    o_out = nc.dram_tensor(
        "o", [B, NHG, NM, 32, 8, 256], F16, kind="ExternalOutput"
    )
    lrw = NH4 * WROW      # 17920 free width of lr tile

    with tile.TileContext(nc) as tc:
        with (
            tc.tile_pool(name="lrpool", bufs=3) as lrp,
            tc.tile_pool(name="gpool", bufs=8) as gp,
            tc.tile_pool(name="ppool", bufs=8, space="PSUM") as pp,
        ):
            lr_tiles = {}
            qw = lrw // 4

            def emit_load(b, frac, nfrac):
                # partial loads (nfrac-th of a b), issued on Pool (SWDGE)
                # which runs ahead of the busy compute/store engines and
                # spread across the previous b's compute -> smooth prefetch.
                if b >= B:
                    return
                if b not in lr_tiles:
                    lr_tiles[b] = lrp.tile([128, lrw], F16, name="lr_t")
                lr_t = lr_tiles[b]
                fw = lrw // nfrac
                nc.gpsimd.dma_start(
                    out=lr_t[:, fw * frac:fw * (frac + 1)],
                    in_=AP(
                        lr_in, b * 2 * C * lrw + fw * frac,
                        [(lrw, 128), (1, fw)],
                    ),
                )

            for et in range(8):
                emit_load(0, et, 8)
            for b in range(B):
                lr_t = lr_tiles[b]
                for hg in range(NHG):
                    emit_load(b + 1, hg, 8)
                    g = gp.tile([128, 4 * GPW], F16, name="g", tag="g")
                    for t in range(4):
                        h4 = 2 * hg + (t >> 1)
                        hh = t & 1
                        cb = h4 * WROW
                        for qp in range(2):
                            # single-bank PSUM tile per q-pair; the 2 matmuls
                            # write it interleaved (col = 2n + qq) so the
                            # eviction is one contiguous f32->f16 copy.
                            # Edge blocks (q=0/q=3) use shortened rhs windows
                            # (the zero pad is dropped from the input); the
                            # uncovered psum slots hold stale data that maps
                            # to out-of-image disparities, zeroed on host.
                            p_t = pp.tile([128, 448], F32, name="p_t")
                            for qq in range(2):
                                q = 2 * qp + qq
                                lhsT = lr_t[
                                    64 * hh:64 * hh + 64,
                                    cb + 128 * q:cb + 128 * q + 128,
                                ]
                                r0 = max(0, 128 * q - MAXD)
                                r1 = min(W, 128 * q + 128 + MAXD)
                                rhs = lr_t[
                                    64 * hh:64 * hh + 64,
                                    cb + W + r0:cb + W + r1,
                                ]
                                # psum col j = 2n + qq, n = r-col - (128q-48)
                                joff = 2 * (r0 - (128 * q - MAXD))
                                nc.tensor.matmul(
                                    AP(
                                        p_t.tensor, qq + joff,
                                        [(448, 128), (2, r1 - r0)],
                                    ),
                                    lhsT, rhs, start=True, stop=True,
                                )
                            tb = 2 * t + qp
                            if qp == 0:
                                nc.vector.tensor_copy(
                                    g[:, 448 * tb:448 * (tb + 1)], p_t[:, :]
                                )
                            else:
                                nc.scalar.copy(
                                    g[:, 448 * tb:448 * (tb + 1)], p_t[:, :]
                                )
                    for m in range(NM):
                        # split store issue between SP and Pool (SWDGE):
                        # each dma_start costs the issuing sequencer ~0.6-1us
                        eng = (
                            nc.sync if (hg * NM + m) % 8 < 3 else nc.gpsimd
                        )
                        eng.dma_start(
                            out=AP(
                                o_out,
                                ((b * NHG + hg) * NM + m) * 32 * 8 * 256,
                                [(8 * 256, 32), (256, 8), (1, 256)],
                            ),
                            in_=AP(
                                g.tensor,
                                m * (32 * 4 * GPW + 64),
                                [(4 * GPW, 32), (448, 8), (1, 256)],
                            ),
                        )
    _split_multi_waits(nc)
    return nc


def _split_multi_waits(nc):
    """The 64-byte TPB instruction encoding holds a single semaphore wait;
    walrus codegen rejects instructions whose sync_info carries more. Hoist
    all but one wait onto standalone InstEventSemaphore instructions placed
    immediately before, on the same engine (FIFO order preserves semantics).
    """
    for bb in nc.main_func.blocks:
        new_list = []
        changed = False
        for ins in bb.instructions:
            si = ins.sync_info
            if si is not None and len(si.on_wait) > 1:
                for w in list(si.on_wait)[:-1]:
                    ev = mybir.InstEventSemaphore(
                        name=nc.get_next_instruction_name(),
                        engine=ins.engine,
                        ins=[],
                        outs=[],
                        sync_info=mybir.SyncInfo(on_wait=[w], on_update=[]),
                    )
                    new_list.append(ev)
                ins.sync_info = mybir.SyncInfo(
                    on_wait=[list(si.on_wait)[-1]], on_update=list(si.on_update)
                )
                changed = True
            new_list.append(ins)
        if changed:
            bb.instructions = new_list


def _get_nc():
    if "nc" not in _NC_CACHE:
        _NC_CACHE["nc"] = _build_nc()
    return _NC_CACHE["nc"]


def _host_prep(l_fmap, r_fmap):
    l = np.asarray(l_fmap, dtype=np.float32) * np.float32(1.0 / C)
    r = np.asarray(r_fmap, dtype=np.float32)
    # per-core layout [k, b, hh, c, h4, col]; h_global = 32k + 2*h4 + hh
    lr = np.empty((NCORES, B, 2, C, NH4, WROW), dtype=np.float16)
    l6 = l.reshape(B, C, NCORES, NH4, 2, W).transpose(2, 0, 4, 1, 3, 5)
    r6 = r.reshape(B, C, NCORES, NH4, 2, W).transpose(2, 0, 4, 1, 3, 5)
    lr[..., 0:W] = l6
    lr[..., W:2 * W] = r6
    return lr


def _install_ntff_hook_shim(so_path="/opt/axon/libaxon_pjrt.so"):
    """Provide antenv.axon_hooks.get_axon_ntff_profile_hook via ctypes when
    the image's antenv lacks it (mirrors trn_agent_boot's slim hook)."""
    import types
    import ctypes
    import contextlib

    try:
        from antenv.axon_hooks import get_axon_ntff_profile_hook  # noqa: F401
        return
    except ImportError:
        pass

    lib = ctypes.CDLL(so_path)
    if not hasattr(lib, "axon_start_nrt_profile"):
        return
    lib.axon_start_nrt_profile.argtypes = [
        ctypes.POINTER(ctypes.c_int64), ctypes.c_size_t,
    ]
    lib.axon_start_nrt_profile.restype = ctypes.c_int64
    lib.axon_stop_nrt_profile.argtypes = [ctypes.c_char_p]
    lib.axon_stop_nrt_profile.restype = ctypes.c_int64

    @contextlib.contextmanager
    def _hook(output_dir, device_ids):
        import jax
        jax.devices()
        if device_ids:
            ids = (ctypes.c_int64 * len(device_ids))(*device_ids)
            rc = lib.axon_start_nrt_profile(ids, len(device_ids))
        else:
            rc = lib.axon_start_nrt_profile(None, 0)
        if rc != 0:
            raise RuntimeError(f"axon_start_nrt_profile rc={rc}")
        try:
            yield
        finally:
            n = lib.axon_stop_nrt_profile(str(output_dir).encode())
            print(f"ntff profile: {n} file(s) written to {output_dir}",
                  file=sys.stderr)

    import antenv
    mod = types.ModuleType("antenv.axon_hooks")
    mod.get_axon_ntff_profile_hook = lambda: _hook
    mod.set_axon_ntff_profile_hook = lambda h: None
    sys.modules["antenv.axon_hooks"] = mod
    antenv.axon_hooks = mod


def kernel(l_fmap, r_fmap, max_disp):
    global LAST_RESULTS
    assert int(max_disp) == MAXD
    lr = _host_prep(l_fmap, r_fmap)

    nc = _get_nc()
    in_maps = [
        {"lr": np.ascontiguousarray(lr[k])} for k in range(NCORES)
    ]

    import os
    trace = bool(int(os.environ.get("CV_TRACE", "0")))
    if trace:
        _install_ntff_hook_shim()
    res = run_bass_kernel_spmd(nc, in_maps, list(range(NCORES)), trace=trace)
    LAST_RESULTS = res

    out = np.empty((B, D, H, W), dtype=np.float32)
    for k in range(NCORES):
        o = np.ascontiguousarray(np.asarray(res.results[k]["o"]))
        s = o.strides  # [B, NHG, NM, 32, 8, 256] f16
        # v9[b, hg, m, i, t, qp, dk, qq] = o[b, hg, m, i, 2t+qp, 2i+2dk+qq]
        v9 = np.lib.stride_tricks.as_strided(
            o,
            shape=(B, NHG, NM, 32, 4, 2, D, 2),
            strides=(s[0], s[1], s[2], s[3] + 2 * s[5], 2 * s[4], s[4],
                     2 * s[5], s[5]),
        )
        # out[b, 96-dk, 32k + 4hg + t, 256qp + 128qq + 32m + i] = v9[...]
        tmp = v9.transpose(0, 6, 1, 4, 5, 7, 2, 3)[:, ::-1]
        out[:, :, HS * k:HS * (k + 1), :] = tmp.reshape(B, D, HS, W)
    # out-of-image disparities (reference zero padding); on-device these
    # slots hold stale PSUM data since the edge matmuls are shortened
    for w in range(MAXD):
        out[:, w + MAXD + 1:, :, w] = 0.0
    for w in range(W - MAXD, W):
        out[:, :w - (W - MAXD - 1), :, w] = 0.0
    return out


# revision 34
# speedup vs baseline: 1.0356x; 1.0356x over previous
"""CostVolume2D Trainium2 kernel (v3, ~114us HW vs 199us baseline).

out[b, d, h, w] = mean_c l[b,c,h,w] * r[b,c,h, w - (d - maxd)]   (zero padded)

Strategy (8 NeuronCores, shard H — no halo since shifts only touch W).
The kernel is HBM/DMA-bound: ~16 DMA engines x ~22.5 B/ns; every design
choice below minimizes bytes moved and keeps descriptors >= 512B.

  * Per (b, h): the 97 disparity planes are the diagonals of banded gram
    blocks G_q[i, n] = sum_c l[c, 128q+i] r[c, 128q-48+n], n in [0,224).
  * Tensor engine: 4 matmuls per h (K=64 channels, M=128 w's, N<=224).
    Each q-PAIR's matmuls write ONE single-bank PSUM tile [128, 448 f32]
    INTERLEAVED (psum col = 2n + qq, out-AP free stride 2). Interleaved
    matmul PSUM writes are correct as long as no write crosses a 2KB
    PSUM bank boundary (2-bank-spanning interleave-4 writes corrupt).
  * Eviction (PSUM -> SBUF f32->f16) is then a plain CONTIGUOUS copy per
    q-pair into g block tb = 2t+qp (448 cols per block, 8 blocks per
    4-h group tile). Split DVE (qp=0) / Activation (qp=1), ~615ns each.
    (Strided DVE/Act WRITES cost 2x — that's why the matmul, not the
    eviction, does the interleaving.)
  * Store: in an interleave-2 block, row i's 194 valid values (97-band
    of a q-pair) sit at cols [2i, 2i+194). For a 32-row sub-block m they
    all fall inside cols [64m, 64m+256) — a plain rectangular slice.
    One 3-dim DMA per (group, m) writes [32 rows x 8 blocks x 256] with
    512B descriptors: 16.8 MB/core vs 29.4 MB for the full-gram store
    (valid data is 12.7 MB). R=32/512B is the optimum under the
    "descriptors < 512B pay 2x" rule + ~10ns fixed cost per descriptor.
  * Input: rows packed [l 512 | r 512] f16, NO zero pad: the edge
    matmuls (q=0/3) use shortened rhs windows (N=176) and the host
    zeroes the out-of-image disparity outputs (stale PSUM) afterwards.
    Each partition (hh, c) reads its 16 h-rows contiguously; loads are
    issued as EIGHTHS on gpsimd (SWDGE, ~25ns dispatch vs ~600ns on
    SP/Act sequencers), one per h-group, prefetching b+1 during b —
    emission order IS issue order per engine, so loads must be emitted
    ahead and spread to avoid saturating DMA in bursts.
  * Store issue is split SP (3/8) / gpsimd (5/8) for the same
    sequencer-cost reason.
  * Host pre-divides l by C (exact, power of two); host unshard is a
    strided view + transpose + edge zeroing (pure layout glue).
"""

import sys

try:
    import concourse  # noqa: F401
except ImportError:
    sys.path.insert(0, "/opt/trn_rl_repo")

import numpy as np

from concourse import bass, mybir
from concourse import tile
from concourse.ap import AP
from concourse.bass_utils import run_bass_kernel_spmd

F32 = mybir.dt.float32
F16 = mybir.dt.float16

# Problem dims (hardcoded per spec)
B, C, H, W = 4, 64, 256, 512
MAXD = 48
D = 2 * MAXD + 1          # 97 disparity planes
NCORES = 8
HS = H // NCORES          # 32 h-rows per core

# Tiling
NH4 = HS // 2             # 16 h-pairs per core (partition dim packs hh in 2)
WROW = 2 * W              # 1024: [l 512 | r 512] per (c, h) row (no zero pad:
                          # edge matmuls are shortened and the host zeroes
                          # the out-of-image disparities afterwards)
NHG = HS // 4             # 8 groups of 4 h per g-tile
GPW = 4 * 224             # 896 interleaved gram columns per h
NM = 4                    # 32-row store sub-blocks
SBW = 512                 # stored row width per sub-block (388 valid + skew)

LAST_RESULTS = None
_NC_CACHE = {}


def _build_nc():
    nc = bass.Bass()
    lr_in = nc.dram_tensor("lr", [B, 2, C, NH4, WROW], F16, kind="ExternalInput")
    o_out = nc.dram_tensor(
        "o", [B, NHG, NM, 32, 8, 256], F16, kind="ExternalOutput"
    )
    lrw = NH4 * WROW      # 17920 free width of lr tile

    with tile.TileContext(nc) as tc:
        with (
            tc.tile_pool(name="lrpool", bufs=3) as lrp,
            tc.tile_pool(name="gpool", bufs=8) as gp,
            tc.tile_pool(name="ppool", bufs=8, space="PSUM") as pp,
        ):
            lr_tiles = {}
            qw = lrw // 4

            def emit_load(b, frac, nfrac):
                # partial loads (nfrac-th of a b), issued on Pool (SWDGE)
                # which runs ahead of the busy compute/store engines and
                # spread across the previous b's compute -> smooth prefetch.
                if b >= B:
                    return
                if b not in lr_tiles:
                    lr_tiles[b] = lrp.tile([128, lrw], F16, name="lr_t")
                lr_t = lr_tiles[b]
                fw = lrw // nfrac
                nc.gpsimd.dma_start(
                    out=lr_t[:, fw * frac:fw * (frac + 1)],
                    in_=AP(
                        lr_in, b * 2 * C * lrw + fw * frac,
                        [(lrw, 128), (1, fw)],
                    ),
                )

            for et in range(8):
                emit_load(0, et, 8)
            for b in range(B):
                lr_t = lr_tiles[b]
                for hg in range(NHG):
                    emit_load(b + 1, hg, 8)
                    g = gp.tile([128, 4 * GPW], F16, name="g", tag="g")
                    for t in range(4):
                        h4 = 2 * hg + (t >> 1)
                        hh = t & 1
                        cb = h4 * WROW
                        for qp in range(2):
                            # single-bank PSUM tile per q-pair; the 2 matmuls
                            # write it interleaved (col = 2n + qq) so the
                            # eviction is one contiguous f32->f16 copy.
                            # Edge blocks (q=0/q=3) use shortened rhs windows
                            # (the zero pad is dropped from the input); the
                            # uncovered psum slots hold stale data that maps
                            # to out-of-image disparities, zeroed on host.
                            p_t = pp.tile([128, 448], F32, name="p_t")
                            for qq in range(2):
                                q = 2 * qp + qq
                                lhsT = lr_t[
                                    64 * hh:64 * hh + 64,
                                    cb + 128 * q:cb + 128 * q + 128,
                                ]
                                r0 = max(0, 128 * q - MAXD)
                                r1 = min(W, 128 * q + 128 + MAXD)
                                rhs = lr_t[
                                    64 * hh:64 * hh + 64,
                                    cb + W + r0:cb + W + r1,
                                ]
                                # psum col j = 2n + qq, n = r-col - (128q-48)
                                joff = 2 * (r0 - (128 * q - MAXD))
                                nc.tensor.matmul(
                                    AP(
                                        p_t.tensor, qq + joff,
                                        [(448, 128), (2, r1 - r0)],
                                    ),
                                    lhsT, rhs, start=True, stop=True,
                                )
                            tb = 2 * t + qp
                            if qp == 0:
                                nc.vector.tensor_copy(
                                    g[:, 448 * tb:448 * (tb + 1)], p_t[:, :]
                                )
                            else:
                                nc.scalar.copy(
                                    g[:, 448 * tb:448 * (tb + 1)], p_t[:, :]
                                )
                    for m in range(NM):
                        # split store issue between SP and Pool (SWDGE):
                        # each dma_start costs the issuing sequencer ~0.6-1us
                        eng = (
                            nc.sync if (hg * NM + m) % 8 < 3 else nc.gpsimd
                        )
                        eng.dma_start(
                            out=AP(
                                o_out,
                                ((b * NHG + hg) * NM + m) * 32 * 8 * 256,
                                [(8 * 256, 32), (256, 8), (1, 256)],
                            ),
                            in_=AP(
                                g.tensor,
                                m * (32 * 4 * GPW + 64),
                                [(4 * GPW, 32), (448, 8), (1, 256)],
                            ),
                        )
    _split_multi_waits(nc)
    return nc


def _split_multi_waits(nc):
    """The 64-byte TPB instruction encoding holds a single semaphore wait;
    walrus codegen rejects instructions whose sync_info carries more. Hoist
    all but one wait onto standalone InstEventSemaphore instructions placed
    immediately before, on the same engine (FIFO order preserves semantics).
    """
    for bb in nc.main_func.blocks:
        new_list = []
        changed = False
        for ins in bb.instructions:
            si = ins.sync_info
            if si is not None and len(si.on_wait) > 1:
                for w in list(si.on_wait)[:-1]:
                    ev = mybir.InstEventSemaphore(
                        name=nc.get_next_instruction_name(),
                        engine=ins.engine,
                        ins=[],
                        outs=[],
                        sync_info=mybir.SyncInfo(on_wait=[w], on_update=[]),
                    )
                    new_list.append(ev)
                ins.sync_info = mybir.SyncInfo(
                    on_wait=[list(si.on_wait)[-1]], on_update=list(si.on_update)
                )
                changed = True
            new_list.append(ins)
        if changed:
            bb.instructions = new_list


def _get_nc():
    if "nc" not in _NC_CACHE:
        _NC_CACHE["nc"] = _build_nc()
    return _NC_CACHE["nc"]


def _host_prep(l_fmap, r_fmap):
    l = np.asarray(l_fmap, dtype=np.float32) * np.float32(1.0 / C)
    r = np.asarray(r_fmap, dtype=np.float32)
    # per-core layout [k, b, hh, c, h4, col]; h_global = 32k + 2*h4 + hh
    lr = np.empty((NCORES, B, 2, C, NH4, WROW), dtype=np.float16)
    l6 = l.reshape(B, C, NCORES, NH4, 2, W).transpose(2, 0, 4, 1, 3, 5)
    r6 = r.reshape(B, C, NCORES, NH4, 2, W).transpose(2, 0, 4, 1, 3, 5)
    lr[..., 0:W] = l6
    lr[..., W:2 * W] = r6
    return lr


def _install_ntff_hook_shim(so_path="/opt/axon/libaxon_pjrt.so"):
    """Provide antenv.axon_hooks.get_axon_ntff_profile_hook via ctypes when
    the image's antenv lacks it (mirrors trn_agent_boot's slim hook)."""
    import types
    import ctypes
    import contextlib

    try:
        from antenv.axon_hooks import get_axon_ntff_profile_hook  # noqa: F401
        return
    except ImportError:
        pass

    lib = ctypes.CDLL(so_path)
    if not hasattr(lib, "axon_start_nrt_profile"):
        return
    lib.axon_start_nrt_profile.argtypes = [
        ctypes.POINTER(ctypes.c_int64), ctypes.c_size_t,
    ]
    lib.axon_start_nrt_profile.restype = ctypes.c_int64
    lib.axon_stop_nrt_profile.argtypes = [ctypes.c_char_p]
    lib.axon_stop_nrt_profile.restype = ctypes.c_int64

    @contextlib.contextmanager
    def _hook(output_dir, device_ids):
        import jax
        jax.devices()
        if device_ids:
            ids = (ctypes.c_int64 * len(device_ids))(*device_ids)
            rc = lib.axon_start_nrt_profile(ids, len(device_ids))
        else:
            rc = lib.axon_start_nrt_profile(None, 0)
        if rc != 0:
            raise RuntimeError(f"axon_start_nrt_profile rc={rc}")
        try:
            yield
        finally:
            n = lib.axon_stop_nrt_profile(str(output_dir).encode())
            print(f"ntff profile: {n} file(s) written to {output_dir}",
                  file=sys.stderr)

    import antenv
    mod = types.ModuleType("antenv.axon_hooks")
    mod.get_axon_ntff_profile_hook = lambda: _hook
    mod.set_axon_ntff_profile_hook = lambda h: None
    sys.modules["antenv.axon_hooks"] = mod
    antenv.axon_hooks = mod


def kernel(l_fmap, r_fmap, max_disp):
    global LAST_RESULTS
    assert int(max_disp) == MAXD
    lr = _host_prep(l_fmap, r_fmap)

    nc = _get_nc()
    in_maps = [
        {"lr": np.ascontiguousarray(lr[k])} for k in range(NCORES)
    ]

    import os
    trace = bool(int(os.environ.get("CV_TRACE", "0")))
    if trace:
        _install_ntff_hook_shim()
    res = run_bass_kernel_spmd(nc, in_maps, list(range(NCORES)), trace=trace)
    LAST_RESULTS = res

    out = np.empty((B, D, H, W), dtype=np.float32)
    for k in range(NCORES):
        o = np.ascontiguousarray(np.asarray(res.results[k]["o"]))
        s = o.strides  # [B, NHG, NM, 32, 8, 256] f16
        # v9[b, hg, m, i, t, qp, dk, qq] = o[b, hg, m, i, 2t+qp, 2i+2dk+qq]
        v9 = np.lib.stride_tricks.as_strided(
            o,
            shape=(B, NHG, NM, 32, 4, 2, D, 2),
            strides=(s[0], s[1], s[2], s[3] + 2 * s[5], 2 * s[4], s[4],
                     2 * s[5], s[5]),
        )
        # out[b, 96-dk, 32k + 4hg + t, 256qp + 128qq + 32m + i] = v9[...]
        tmp = v9.transpose(0, 6, 1, 4, 5, 7, 2, 3)[:, ::-1]
        out[:, :, HS * k:HS * (k + 1), :] = tmp.reshape(B, D, HS, W)
    # out-of-image disparities (reference zero padding); on-device these
    # slots hold stale PSUM data since the edge matmuls are shortened
    for w in range(MAXD):
        out[:, w + MAXD + 1:, :, w] = 0.0
    for w in range(W - MAXD, W):
        out[:, :w - (W - MAXD - 1), :, w] = 0.0
    return out


# revision 35
# speedup vs baseline: 1.0366x; 1.0010x over previous
"""CostVolume2D Trainium2 kernel (v3, ~114us HW vs 199us baseline).

out[b, d, h, w] = mean_c l[b,c,h,w] * r[b,c,h, w - (d - maxd)]   (zero padded)

Strategy (8 NeuronCores, shard H — no halo since shifts only touch W).
The kernel is HBM/DMA-bound: ~16 DMA engines x ~22.5 B/ns; every design
choice below minimizes bytes moved and keeps descriptors >= 512B.

  * Per (b, h): the 97 disparity planes are the diagonals of banded gram
    blocks G_q[i, n] = sum_c l[c, 128q+i] r[c, 128q-48+n], n in [0,224).
  * Tensor engine: 4 matmuls per h (K=64 channels, M=128 w's, N<=224).
    Each q-PAIR's matmuls write ONE single-bank PSUM tile [128, 448 f32]
    INTERLEAVED (psum col = 2n + qq, out-AP free stride 2). Interleaved
    matmul PSUM writes are correct as long as no write crosses a 2KB
    PSUM bank boundary (2-bank-spanning interleave-4 writes corrupt).
  * Eviction (PSUM -> SBUF f32->f16) is then a plain CONTIGUOUS copy per
    q-pair into g block tb = 2t+qp (448 cols per block, 8 blocks per
    4-h group tile). Split DVE (qp=0) / Activation (qp=1), ~615ns each.
    (Strided DVE/Act WRITES cost 2x — that's why the matmul, not the
    eviction, does the interleaving.)
  * Store: in an interleave-2 block, row i's 194 valid values (97-band
    of a q-pair) sit at cols [2i, 2i+194). For a 32-row sub-block m they
    all fall inside cols [64m, 64m+256) — a plain rectangular slice.
    One 3-dim DMA per (group, m) writes [32 rows x 8 blocks x 256] with
    512B descriptors: 16.8 MB/core vs 29.4 MB for the full-gram store
    (valid data is 12.7 MB). R=32/512B is the optimum under the
    "descriptors < 512B pay 2x" rule + ~10ns fixed cost per descriptor.
  * Input: rows packed [l 512 | r 512] f16, NO zero pad: the edge
    matmuls (q=0/3) use shortened rhs windows (N=176) and the host
    zeroes the out-of-image disparity outputs (stale PSUM) afterwards.
    Each partition (hh, c) reads its 16 h-rows contiguously; loads are
    issued as EIGHTHS on gpsimd (SWDGE, ~25ns dispatch vs ~600ns on
    SP/Act sequencers), one per h-group, prefetching b+1 during b —
    emission order IS issue order per engine, so loads must be emitted
    ahead and spread to avoid saturating DMA in bursts.
  * Store issue is split SP (3/8) / gpsimd (5/8) for the same
    sequencer-cost reason.
  * Host pre-divides l by C (exact, power of two); host unshard is a
    strided view + transpose + edge zeroing (pure layout glue).
"""

import sys

try:
    import concourse  # noqa: F401
except ImportError:
    sys.path.insert(0, "/opt/trn_rl_repo")

import numpy as np

from concourse import bass, mybir
from concourse import tile
from concourse.ap import AP
from concourse.bass_utils import run_bass_kernel_spmd

F32 = mybir.dt.float32
F16 = mybir.dt.float16

# Problem dims (hardcoded per spec)
B, C, H, W = 4, 64, 256, 512
MAXD = 48
D = 2 * MAXD + 1          # 97 disparity planes
NCORES = 8
HS = H // NCORES          # 32 h-rows per core

# Tiling
NH4 = HS // 2             # 16 h-pairs per core (partition dim packs hh in 2)
WROW = 2 * W              # 1024: [l 512 | r 512] per (c, h) row (no zero pad:
                          # edge matmuls are shortened and the host zeroes
                          # the out-of-image disparities afterwards)
NHG = HS // 4             # 8 groups of 4 h per g-tile
GPW = 4 * 224             # 896 interleaved gram columns per h
NM = 4                    # 32-row store sub-blocks
SBW = 512                 # stored row width per sub-block (388 valid + skew)

LAST_RESULTS = None
_NC_CACHE = {}


def _build_nc():
    nc = bass.Bass()
    lr_in = nc.dram_tensor("lr", [B, 2, C, NH4, WROW], F16, kind="ExternalInput")
    o_out = nc.dram_tensor(
        "o", [B, NHG, NM, 32, 8, 256], F16, kind="ExternalOutput"
    )
    lrw = NH4 * WROW      # 17920 free width of lr tile

    with tile.TileContext(nc) as tc:
        with (
            tc.tile_pool(name="lrpool", bufs=3) as lrp,
            tc.tile_pool(name="gpool", bufs=8) as gp,
            tc.tile_pool(name="ppool", bufs=8, space="PSUM") as pp,
        ):
            lr_tiles = {}
            qw = lrw // 4

            def emit_load(b, frac, nfrac):
                # partial loads (nfrac-th of a b), issued on Pool (SWDGE)
                # which runs ahead of the busy compute/store engines and
                # spread across the previous b's compute -> smooth prefetch.
                if b >= B:
                    return
                if b not in lr_tiles:
                    lr_tiles[b] = lrp.tile([128, lrw], F16, name="lr_t")
                lr_t = lr_tiles[b]
                fw = lrw // nfrac
                nc.gpsimd.dma_start(
                    out=lr_t[:, fw * frac:fw * (frac + 1)],
                    in_=AP(
                        lr_in, b * 2 * C * lrw + fw * frac,
                        [(lrw, 128), (1, fw)],
                    ),
                )

            for et in range(8):
                emit_load(0, et, 8)
            for b in range(B):
                lr_t = lr_tiles[b]
                for hg in range(NHG):
                    emit_load(b + 1, hg, 8)
                    g = gp.tile([128, 4 * GPW], F16, name="g", tag="g")
                    for t in range(4):
                        h4 = 2 * hg + (t >> 1)
                        hh = t & 1
                        cb = h4 * WROW
                        for qp in range(2):
                            # single-bank PSUM tile per q-pair; the 2 matmuls
                            # write it interleaved (col = 2n + qq) so the
                            # eviction is one contiguous f32->f16 copy.
                            # Edge blocks (q=0/q=3) use shortened rhs windows
                            # (the zero pad is dropped from the input); the
                            # uncovered psum slots hold stale data that maps
                            # to out-of-image disparities, zeroed on host.
                            p_t = pp.tile([128, 448], F32, name="p_t")
                            for qq in range(2):
                                q = 2 * qp + qq
                                lhsT = lr_t[
                                    64 * hh:64 * hh + 64,
                                    cb + 128 * q:cb + 128 * q + 128,
                                ]
                                r0 = max(0, 128 * q - MAXD)
                                r1 = min(W, 128 * q + 128 + MAXD)
                                rhs = lr_t[
                                    64 * hh:64 * hh + 64,
                                    cb + W + r0:cb + W + r1,
                                ]
                                # psum col j = 2n + qq, n = r-col - (128q-48)
                                joff = 2 * (r0 - (128 * q - MAXD))
                                nc.tensor.matmul(
                                    AP(
                                        p_t.tensor, qq + joff,
                                        [(448, 128), (2, r1 - r0)],
                                    ),
                                    lhsT, rhs, start=True, stop=True,
                                )
                            tb = 2 * t + qp
                            if qp == 0:
                                nc.vector.tensor_copy(
                                    g[:, 448 * tb:448 * (tb + 1)], p_t[:, :]
                                )
                            else:
                                nc.scalar.copy(
                                    g[:, 448 * tb:448 * (tb + 1)], p_t[:, :]
                                )
                    for m in range(NM):
                        # split store issue between SP and Pool (SWDGE):
                        # each dma_start costs the issuing sequencer ~0.6-1us
                        eng = (
                            nc.sync if (hg * NM + m) % 2 == 0 else nc.gpsimd
                        )
                        eng.dma_start(
                            out=AP(
                                o_out,
                                ((b * NHG + hg) * NM + m) * 32 * 8 * 256,
                                [(8 * 256, 32), (256, 8), (1, 256)],
                            ),
                            in_=AP(
                                g.tensor,
                                m * (32 * 4 * GPW + 64),
                                [(4 * GPW, 32), (448, 8), (1, 256)],
                            ),
                        )
    _split_multi_waits(nc)
    return nc


def _split_multi_waits(nc):
    """The 64-byte TPB instruction encoding holds a single semaphore wait;
    walrus codegen rejects instructions whose sync_info carries more. Hoist
    all but one wait onto standalone InstEventSemaphore instructions placed
    immediately before, on the same engine (FIFO order preserves semantics).
    """
    for bb in nc.main_func.blocks:
        new_list = []
        changed = False
        for ins in bb.instructions:
            si = ins.sync_info
            if si is not None and len(si.on_wait) > 1:
                for w in list(si.on_wait)[:-1]:
                    ev = mybir.InstEventSemaphore(
                        name=nc.get_next_instruction_name(),
                        engine=ins.engine,
                        ins=[],
                        outs=[],
                        sync_info=mybir.SyncInfo(on_wait=[w], on_update=[]),
                    )
                    new_list.append(ev)
                ins.sync_info = mybir.SyncInfo(
                    on_wait=[list(si.on_wait)[-1]], on_update=list(si.on_update)
                )
                changed = True
            new_list.append(ins)
        if changed:
            bb.instructions = new_list


def _get_nc():
    if "nc" not in _NC_CACHE:
        _NC_CACHE["nc"] = _build_nc()
    return _NC_CACHE["nc"]


def _host_prep(l_fmap, r_fmap):
    l = np.asarray(l_fmap, dtype=np.float32) * np.float32(1.0 / C)
    r = np.asarray(r_fmap, dtype=np.float32)
    # per-core layout [k, b, hh, c, h4, col]; h_global = 32k + 2*h4 + hh
    lr = np.empty((NCORES, B, 2, C, NH4, WROW), dtype=np.float16)
    l6 = l.reshape(B, C, NCORES, NH4, 2, W).transpose(2, 0, 4, 1, 3, 5)
    r6 = r.reshape(B, C, NCORES, NH4, 2, W).transpose(2, 0, 4, 1, 3, 5)
    lr[..., 0:W] = l6
    lr[..., W:2 * W] = r6
    return lr


def _install_ntff_hook_shim(so_path="/opt/axon/libaxon_pjrt.so"):
    """Provide antenv.axon_hooks.get_axon_ntff_profile_hook via ctypes when
    the image's antenv lacks it (mirrors trn_agent_boot's slim hook)."""
    import types
    import ctypes
    import contextlib

    try:
        from antenv.axon_hooks import get_axon_ntff_profile_hook  # noqa: F401
        return
    except ImportError:
        pass

    lib = ctypes.CDLL(so_path)
    if not hasattr(lib, "axon_start_nrt_profile"):
        return
    lib.axon_start_nrt_profile.argtypes = [
        ctypes.POINTER(ctypes.c_int64), ctypes.c_size_t,
    ]
    lib.axon_start_nrt_profile.restype = ctypes.c_int64
    lib.axon_stop_nrt_profile.argtypes = [ctypes.c_char_p]
    lib.axon_stop_nrt_profile.restype = ctypes.c_int64

    @contextlib.contextmanager
    def _hook(output_dir, device_ids):
        import jax
        jax.devices()
        if device_ids:
            ids = (ctypes.c_int64 * len(device_ids))(*device_ids)
            rc = lib.axon_start_nrt_profile(ids, len(device_ids))
        else:
            rc = lib.axon_start_nrt_profile(None, 0)
        if rc != 0:
            raise RuntimeError(f"axon_start_nrt_profile rc={rc}")
        try:
            yield
        finally:
            n = lib.axon_stop_nrt_profile(str(output_dir).encode())
            print(f"ntff profile: {n} file(s) written to {output_dir}",
                  file=sys.stderr)

    import antenv
    mod = types.ModuleType("antenv.axon_hooks")
    mod.get_axon_ntff_profile_hook = lambda: _hook
    mod.set_axon_ntff_profile_hook = lambda h: None
    sys.modules["antenv.axon_hooks"] = mod
    antenv.axon_hooks = mod


def kernel(l_fmap, r_fmap, max_disp):
    global LAST_RESULTS
    assert int(max_disp) == MAXD
    lr = _host_prep(l_fmap, r_fmap)

    nc = _get_nc()
    in_maps = [
        {"lr": np.ascontiguousarray(lr[k])} for k in range(NCORES)
    ]

    import os
    trace = bool(int(os.environ.get("CV_TRACE", "0")))
    if trace:
        _install_ntff_hook_shim()
    res = run_bass_kernel_spmd(nc, in_maps, list(range(NCORES)), trace=trace)
    LAST_RESULTS = res

    out = np.empty((B, D, H, W), dtype=np.float32)
    for k in range(NCORES):
        o = np.ascontiguousarray(np.asarray(res.results[k]["o"]))
        s = o.strides  # [B, NHG, NM, 32, 8, 256] f16
        # v9[b, hg, m, i, t, qp, dk, qq] = o[b, hg, m, i, 2t+qp, 2i+2dk+qq]
        v9 = np.lib.stride_tricks.as_strided(
            o,
            shape=(B, NHG, NM, 32, 4, 2, D, 2),
            strides=(s[0], s[1], s[2], s[3] + 2 * s[5], 2 * s[4], s[4],
                     2 * s[5], s[5]),
        )
        # out[b, 96-dk, 32k + 4hg + t, 256qp + 128qq + 32m + i] = v9[...]
        tmp = v9.transpose(0, 6, 1, 4, 5, 7, 2, 3)[:, ::-1]
        out[:, :, HS * k:HS * (k + 1), :] = tmp.reshape(B, D, HS, W)
    # out-of-image disparities (reference zero padding); on-device these
    # slots hold stale PSUM data since the edge matmuls are shortened
    for w in range(MAXD):
        out[:, w + MAXD + 1:, :, w] = 0.0
    for w in range(W - MAXD, W):
        out[:, :w - (W - MAXD - 1), :, w] = 0.0
    return out
